# revision 8
# baseline (speedup 1.0000x reference)
"""Trainium2 Bass kernel for nn_AttentionBlock (B=4, H=W=64, C=256, D=32).

Sharding: 8 shards = 4 samples x 2 query-halves. Each core gets the full
sample's rows (reordered so its 2048 query rows come first), computes K for
all 4096 keys, and attention for its 2048 queries. No collectives.

v3 algorithm (projection folding + fp8 end-to-end):
  out = x + (1/d) * (G^T @ W2),  W2 = 32 * wv @ wo   (host precompute)
  G[c,q] = sum_k x8[k,c] * E8[k,q]   (fp8 DoubleRow matmuls, contraction 256)
  E8 = fp8e5m2(exp(S/256 - 2)),  S = K Q^T scores at x256 scale (wq,wk
       stored e4m3 with x16 scale each; compensated in the exp scale)
  d[q] = sum_k E8[k,q]       (col-packed ones matmuls + transpose matmul)
exp computed two ways in parallel: ACT true exp -> e5m2, and DVE integer
bit-trick (Schraudolph in e5m2 space: bits = (5.7708/256)*S + 48.76).
S matmuls (contraction D=32) use 4-way PE row-tiling via tile_position.
HAM warmup matmuls at start keep the PE clock at 2.4GHz.

Self-contained: hardcodes shapes, imports only /opt/trn_rl_repo concourse.
"""

import sys

if "/opt/trn_rl_repo" not in sys.path:
    sys.path.insert(0, "/opt/trn_rl_repo")

import numpy as np
import ml_dtypes

BF16 = ml_dtypes.bfloat16
E4M3 = ml_dtypes.float8_e4m3
E5M2 = ml_dtypes.float8_e5m2

# Problem constants
B, HH, WW, C = 4, 64, 64, 256
D = 32
N = HH * WW           # 4096 keys per sample
NQ = N // 2           # 2048 queries per core
NCORES = 8
KC = N // 128         # 32 key chunks
NG = NQ // 512        # 4 query groups of 512 per core
NSTEP = 8             # 4-chunk steps per query group (32 chunks / 4)
PIPE = 3              # consume s - PIPE

C0 = 2.0              # exp shift: weights = exp(S - C0), cancels in softmax
SS = 256.0            # scores arrive at x256 scale (wq,wk each x16 in fp8)
EXP_A = 5.770780 / SS  # 4*log2(e) / 256
EXP_B = 60.0 + 0.3 - 5.770780 * C0  # e5m2 bias 60, +0.3 truncation recenter
NWARM = 40            # HAM warmup matmuls

_compiled_cache = {}


def _build():
    from contextlib import ExitStack
    from concourse import bacc, tile, mybir

    f32 = mybir.dt.float32
    bf = mybir.dt.bfloat16
    fp8e4 = mybir.dt.float8e4
    fp8e5 = mybir.dt.float8e5
    u8 = mybir.dt.uint8

    nc = bacc.Bacc("TRN2", target_bir_lowering=False, debug=False, num_devices=NCORES)

    x8_d = nc.dram_tensor("x8", [N, C], fp8e4, kind="ExternalInput")
    xq32_d = nc.dram_tensor("xq32", [NQ, C], f32, kind="ExternalInput")
    wqa_d = nc.dram_tensor("wqa_rep", [256, 128], fp8e4, kind="ExternalInput")
    wka_d = nc.dram_tensor("wka_rep", [256, 128], fp8e4, kind="ExternalInput")
    bq_d = nc.dram_tensor("bq_col", [128, 1], f32, kind="ExternalInput")
    bk_d = nc.dram_tensor("bk_col", [128, 1], f32, kind="ExternalInput")
    w2_d = nc.dram_tensor("w2", [256, 256], bf, kind="ExternalInput")
    id_d = nc.dram_tensor("ident8", [128, 128], fp8e4, kind="ExternalInput")
    out_d = nc.dram_tensor("out", [NQ, C], f32, kind="ExternalOutput")

    Exp = mybir.ActivationFunctionType.Exp
    Add = mybir.AluOpType.add
    Mult = mybir.AluOpType.mult
    DR = mybir.MatmulPerfMode.DoubleRow

    with tile.TileContext(nc) as tc:
        with ExitStack() as ctx:
            const = ctx.enter_context(tc.tile_pool(name="const", bufs=1))
            big = ctx.enter_context(tc.tile_pool(name="big", bufs=1))
            expp = ctx.enter_context(tc.tile_pool(name="expp", bufs=8))
            small = ctx.enter_context(tc.tile_pool(name="small", bufs=2))
            ps_s = ctx.enter_context(tc.tile_pool(name="ps_s", bufs=2, space="PSUM"))
            ps_g = ctx.enter_context(tc.tile_pool(name="ps_g", bufs=1, space="PSUM"))
            ps_d = ctx.enter_context(tc.tile_pool(name="ps_d", bufs=1, space="PSUM"))
            ps_e = ctx.enter_context(tc.tile_pool(name="ps_e", bufs=1, space="PSUM"))

            # ---- input DMAs: big x8 first (sync+scalar rings), weights on scalar ----
            x8sb = big.tile([128, KC, 256], fp8e4, tag="x8sb")
            x8_r = x8_d[:].rearrange("(t p) c -> p t c", p=128)
            ident = const.tile([128, 128], fp8e4, tag="ident")
            wq0 = const.tile([128, 128], fp8e4, tag="wq0")
            wq1 = const.tile([128, 128], fp8e4, tag="wq1")
            wk0 = const.tile([128, 128], fp8e4, tag="wk0")
            wk1 = const.tile([128, 128], fp8e4, tag="wk1")
            w2sb = const.tile([128, 2, 256], bf, tag="w2sb")
            bqc = const.tile([128, 1], f32, tag="bqc")
            bkc = const.tile([128, 1], f32, tag="bkc")
            xqg = big.tile([128, 16, 256], f32, tag="xqg")
            xq_r = xq32_d[:].rearrange("(t p) c -> p t c", p=128)

            for d in range(8):
                eng = nc.sync if d % 2 == 0 else nc.scalar
                eng.dma_start(out=x8sb[:, 4 * d : 4 * d + 4, :], in_=x8_r[:, 4 * d : 4 * d + 4, :])
            nc.scalar.dma_start(out=ident[:], in_=id_d[:])
            nc.scalar.dma_start(out=wq0[:], in_=wqa_d[0:128, :])
            nc.scalar.dma_start(out=wq1[:], in_=wqa_d[128:256, :])
            nc.scalar.dma_start(out=wk0[:], in_=wka_d[0:128, :])
            nc.scalar.dma_start(out=wk1[:], in_=wka_d[128:256, :])
            nc.scalar.dma_start(out=w2sb[:, 0, :], in_=w2_d[0:128, :])
            nc.scalar.dma_start(out=w2sb[:, 1, :], in_=w2_d[128:256, :])
            nc.scalar.dma_start(out=bqc[:], in_=bq_d[:])
            nc.scalar.dma_start(out=bkc[:], in_=bk_d[:])
            nc.sync.dma_start(out=xqg[:, 0:8, :], in_=xq_r[:, 0:8, :])
            nc.sync.dma_start(out=xqg[:, 8:16, :], in_=xq_r[:, 8:16, :])

            ones8 = const.tile([128, 32], fp8e5, tag="ones8")
            nc.gpsimd.memset(ones8[:], 1.0)
            ones1 = const.tile([128, 1], bf, tag="ones1")
            nc.gpsimd.memset(ones1[:], 1.0)
            negc0 = const.tile([128, 1], f32, tag="negc0")
            nc.gpsimd.memset(negc0[:], -C0)

            # ---- HAM warmup: dense dummy matmuls while DMAs land ----
            wmt = ps_s.tile([128, 2, 512], f32, tag="s", name="warm")
            for i in range(NWARM):
                nc.tensor.matmul(wmt[:, 0, 0:128], ident[:], ident[:], start=True, stop=True)
            # pre-load ACT exp table
            dumm = const.tile([128, 1], f32, tag="dumm")
            nc.scalar.activation(dumm[:], negc0[:], Exp, bias=negc0[:])

            # ---- phase A: x8 -> xT8 (channel-major), via identity matmuls ----
            xT8 = big.tile([128, 2, N], fp8e4, tag="xT8")  # [:, h, :]: ch 128h..128h+127
            for t in range(16):
                ta, tb = 2 * t, 2 * t + 1
                pt = ps_s.tile([128, 2, 512], f32, tag="s")
                nc.tensor.matmul(pt[:, 0, 0:128], x8sb[:, ta, 0:128], ident[:], start=True, stop=True)
                nc.tensor.matmul(pt[:, 0, 128:256], x8sb[:, tb, 0:128], ident[:], start=True, stop=True)
                nc.tensor.matmul(pt[:, 1, 0:128], x8sb[:, ta, 128:256], ident[:], start=True, stop=True)
                nc.tensor.matmul(pt[:, 1, 128:256], x8sb[:, tb, 128:256], ident[:], start=True, stop=True)
                nc.vector.tensor_copy(xT8[:, 0, 256 * t : 256 * t + 256], pt[:, 0, 0:256])
                nc.vector.tensor_copy(xT8[:, 1, 256 * t : 256 * t + 256], pt[:, 1, 0:256])

            # ---- phase B: qT/kT replicated x4 along partitions (d = 32 each) ----
            qT = big.tile([128, NQ], bf, tag="qT")
            kT = big.tile([128, N], bf, tag="kT")
            for s in range(NQ // 512):
                pq = ps_s.tile([128, 2, 512], f32, tag="s")
                nc.tensor.matmul(pq[:, 0, :], wq0[:], xT8[:, 0, 512 * s : 512 * s + 512], start=True, stop=False)
                nc.tensor.matmul(pq[:, 0, :], wq1[:], xT8[:, 1, 512 * s : 512 * s + 512], start=False, stop=True)
                nc.vector.tensor_scalar(qT[:, 512 * s : 512 * s + 512], pq[:, 0, :], bqc[:], None, Add)
            for s in range(N // 512):
                pk = ps_s.tile([128, 2, 512], f32, tag="s")
                nc.tensor.matmul(pk[:, 0, :], wk0[:], xT8[:, 0, 512 * s : 512 * s + 512], start=True, stop=False)
                nc.tensor.matmul(pk[:, 0, :], wk1[:], xT8[:, 1, 512 * s : 512 * s + 512], start=False, stop=True)
                nc.vector.tensor_scalar(kT[:, 512 * s : 512 * s + 512], pk[:, 0, :], bkc[:], None, Add)

            # ---- phase C: flat pipeline over 32 steps of 4 key chunks ----
            sts = {}
            ets = {}
            gtile = {}
            dtile = {}
            ottile = {}

            def produce(s):
                g, t = divmod(s, NSTEP)
                if t == 0:
                    gtile[g] = ps_g.tile([128, 2, 512], f32, tag="g", name=f"g{g}")
                    dtile[g] = ps_d.tile([128, 512], f32, tag="d", name=f"d{g}")
                    ottile[g] = small.tile([128, 4, 256], f32, tag="ot", name=f"ot{g}")
                sA = ps_s.tile([128, 2, 512], f32, tag="s", name=f"sA{s}")
                sB = ps_s.tile([128, 2, 512], f32, tag="s", name=f"sB{s}")
                for i in range(4):
                    m = 4 * t + i
                    dst = (sA if i < 2 else sB)[:, i % 2, :]
                    nc.tensor.matmul(
                        dst,
                        kT[32 * i : 32 * i + 32, 128 * m : 128 * m + 128],
                        qT[32 * i : 32 * i + 32, 512 * g : 512 * g + 512],
                        start=True,
                        stop=True,
                        tile_position=(32 * i, 0),
                    )
                etA = expp.tile([128, 2, 512], fp8e5, tag="e", name=f"eA{s}")
                etB = expp.tile([128, 2, 512], fp8e5, tag="e", name=f"eB{s}")
                nc.scalar.activation(etA[:], sA[:], Exp, bias=negc0[:], scale=1.0 / SS)
                nc.vector.tensor_scalar(
                    etB[:].bitcast(u8), sB[:], EXP_A, EXP_B, Mult, Add
                )
                sts[s] = (sA, sB)
                ets[s] = (etA, etB)

            def consume(s):
                g, t = divmod(s, NSTEP)
                etA, etB = ets.pop(s)
                sts.pop(s)
                gp = gtile[g]
                dp = dtile[g]
                for pa, et in ((0, etA), (1, etB)):
                    pp = 2 * t + pa
                    for h in range(2):
                        nc.tensor.matmul(
                            gp[:, h, :],
                            x8sb[:, 4 * t + 2 * pa : 4 * t + 2 * pa + 2, 128 * h : 128 * h + 128],
                            et[:],
                            start=(pp == 0),
                            stop=(pp == 2 * NSTEP - 1),
                            perf_mode=DR,
                        )
                for j in range(4):
                    et = (etA if j < 2 else etB)
                    nc.tensor.matmul(
                        dp[32 * j : 32 * j + 32, :],
                        ones8[:],
                        et[:, j % 2, :],
                        start=(t == 0),
                        stop=(t == NSTEP - 1),
                        tile_position=(0, 32 * j),
                    )

            def group_epilogue(g):
                gp = gtile.pop(g)
                dp = dtile.pop(g)
                ott = ottile.pop(g)
                gsb = small.tile([128, 2, 512], bf, tag="gsb")
                nc.vector.tensor_copy(gsb[:], gp[:])
                dsb = small.tile([128, 512], bf, tag="dsb")
                nc.vector.tensor_copy(dsb[:], dp[:])
                er = ps_e.tile([128, 512], f32, tag="er", name=f"er{g}")
                for b in range(4):
                    nc.tensor.matmul(
                        er[:, 256 + b : 257 + b], dsb[:, 128 * b : 128 * b + 128],
                        ones1[:], start=True, stop=True,
                    )
                rec = small.tile([128, 4], f32, tag="recs")
                nc.vector.reciprocal(rec[:], er[:, 256:260])
                for b in range(4):
                    ep = er[:, 0:256]
                    nc.tensor.matmul(ep, gsb[:, 0, 128 * b : 128 * b + 128], w2sb[:, 0, :], start=True, stop=False)
                    nc.tensor.matmul(ep, gsb[:, 1, 128 * b : 128 * b + 128], w2sb[:, 1, :], start=False, stop=True)
                    nc.vector.scalar_tensor_tensor(
                        ott[:, b, :], ep, rec[:, b : b + 1], xqg[:, 4 * g + b, :], Mult, Add
                    )
                out_r = out_d[:].rearrange("(t p) c -> p t c", p=128)
                nc.sync.dma_start(out=out_r[:, 4 * g : 4 * g + 4, :], in_=ott[:])

            for s in range(NG * NSTEP + PIPE):
                if s >= PIPE:
                    sc = s - PIPE
                    consume(sc)
                    if sc % NSTEP == NSTEP - 1:
                        group_epilogue(sc // NSTEP)
                if s < NG * NSTEP:
                    produce(s)

    nc.compile()
    return nc


def _get_compiled():
    if "v3" not in _compiled_cache:
        _compiled_cache["v3"] = _build()
    return _compiled_cache["v3"]


def _prep(x, wq, bq, wk, bk, wv, bv, wo, bo):
    xf = np.ascontiguousarray(np.asarray(x, dtype=np.float32)).reshape(B, N, C)
    wq = np.asarray(wq, np.float32)
    bq = np.asarray(bq, np.float32)
    wk = np.asarray(wk, np.float32)
    bk = np.asarray(bk, np.float32)
    wv = np.asarray(wv, np.float32)
    bv = np.asarray(bv, np.float32)
    wo = np.asarray(wo, np.float32)
    bo = np.asarray(bo, np.float32)

    scale = np.float32(1.0 / np.sqrt(np.float32(D)))
    # wq,wk stored e4m3 at x16 scale each (S comes out x256; exp rescales)
    wqa_rep = np.ascontiguousarray(np.tile(wq * (scale * 16.0), (1, 4))).astype(E4M3)
    wka_rep = np.ascontiguousarray(np.tile(wk * 16.0, (1, 4))).astype(E4M3)
    bq_col = np.ascontiguousarray(np.tile(bq * (scale * 16.0), 4)[:, None]).astype(np.float32)
    bk_col = np.ascontiguousarray(np.tile(bk * 16.0, 4)[:, None]).astype(np.float32)
    w2 = np.ascontiguousarray(32.0 * (wv @ wo)).astype(BF16)  # x32 folds denom replication
    r0 = (bv @ wo + bo).astype(np.float32)  # residual-folded constant bias row
    ident8 = np.eye(128, dtype=np.float32).astype(E4M3)

    in_maps = []
    for core in range(NCORES):
        b, h = divmod(core, 2)
        if h == 0:
            xo = xf[b]
        else:
            xo = np.concatenate([xf[b, NQ:], xf[b, :NQ]], 0)
        in_maps.append(
            {
                "x8": xo.astype(E4M3),
                "xq32": np.ascontiguousarray(xo[:NQ] + r0[None, :]),
                "wqa_rep": wqa_rep,
                "wka_rep": wka_rep,
                "bq_col": bq_col,
                "bk_col": bk_col,
                "w2": w2,
                "ident8": ident8,
            }
        )
    return in_maps


def _gather(results):
    out = np.empty((B, N, C), np.float32)
    for core in range(NCORES):
        b, h = divmod(core, 2)
        out[b, NQ * h : NQ * (h + 1)] = results[core]["out"]
    return out.reshape(B, HH, WW, C)


def kernel(x, wq, bq, wk, bk, wv, bv, wo, bo):
    from concourse.bass_utils import run_bass_kernel_spmd

    in_maps = _prep(x, wq, bq, wk, bk, wv, bv, wo, bo)
    nc = _get_compiled()
    res = run_bass_kernel_spmd(nc, in_maps, core_ids=list(range(NCORES)))
    return _gather(res.results)


def _ensure_ntff_hook():
    """The agent image's antenv stub lacks axon_hooks; synthesize it so
    run_bass_kernel_spmd(trace=True) can NTFF-profile via libaxon_pjrt."""
    import types

    try:
        from antenv.axon_hooks import get_axon_ntff_profile_hook  # noqa: F401
        return
    except ImportError:
        pass
    import antenv
    from trn_agent_boot.trn_boot import _ntff_profile_via_ctypes

    mod = types.ModuleType("antenv.axon_hooks")
    state = {"h": _ntff_profile_via_ctypes("/opt/axon/libaxon_pjrt.so")}
    mod.get_axon_ntff_profile_hook = lambda: state["h"]
    mod.set_axon_ntff_profile_hook = lambda h: state.__setitem__("h", h)
    sys.modules["antenv.axon_hooks"] = mod
    antenv.axon_hooks = mod


def run_traced(inputs, **kw):
    """For test.py: run with NTFF profiling; returns (output, BassKernelResults)."""
    from concourse.bass_utils import run_bass_kernel_spmd

    _ensure_ntff_hook()

    in_maps = _prep(**inputs)
    nc = _get_compiled()
    res = run_bass_kernel_spmd(nc, in_maps, core_ids=list(range(NCORES)), trace=True, **kw)
    return _gather(res.results), res


# revision 9
# speedup vs baseline: 1.3129x; 1.3129x over previous
"""Trainium2 Bass kernel for nn_AttentionBlock (B=4, H=W=64, C=256, D=32).

Sharding: 8 shards = 4 samples x 2 query-halves. Each core gets the full
sample's rows (reordered so its 2048 query rows come first), computes K for
all 4096 keys, and attention for its 2048 queries. No collectives.

v4 algorithm (projection folding + fp8 end-to-end):
  out = x + (1/d) * (G^T @ W2),  W2 = 32 * wv @ wo   (host precompute)
  G[c,q] = sum_k x8[k,c] * E8[k,q]   (fp8 DoubleRow matmuls, contraction 256)
  E8 = fp8e5m2(exp(S/256 - 2)),  S = K Q^T scores at x256 scale (wq,wk
       stored e4m3 with x16 scale each; compensated in the exp scale)
  d[q] = sum_k E8[k,q]       (col-packed ones matmuls + transpose matmul)
exp computed two ways in parallel: ACT true exp -> e5m2, and DVE integer
bit-trick (Schraudolph in e5m2 space: bits = (5.7708/256)*S + 48.76).
S matmuls (contraction D=32) use 4-way PE row-tiling via tile_position.
Q/K projections use DoubleRow. HAM warmup matmuls bridge the ~9us DMA
startup dead zone so the PE clock is at 2.4GHz when real work arrives.
Host supplies x8/xq32 in partition-major layout for fat DMA descriptors.

Self-contained: hardcodes shapes, imports only /opt/trn_rl_repo concourse.
"""

import sys

if "/opt/trn_rl_repo" not in sys.path:
    sys.path.insert(0, "/opt/trn_rl_repo")

import numpy as np
import ml_dtypes

BF16 = ml_dtypes.bfloat16
E4M3 = ml_dtypes.float8_e4m3
E5M2 = ml_dtypes.float8_e5m2

# Problem constants
B, HH, WW, C = 4, 64, 64, 256
D = 32
N = HH * WW           # 4096 keys per sample
NQ = N // 2           # 2048 queries per core
NCORES = 8
KC = N // 128         # 32 key chunks
NG = NQ // 512        # 4 query groups of 512 per core
NSTEP = 8             # 4-chunk steps per query group (32 chunks / 4)
PIPE = 3              # consume s - PIPE

C0 = 2.0              # exp shift: weights = exp(S - C0), cancels in softmax
SS = 256.0            # scores arrive at x256 scale (wq,wk each x16 in fp8)
EXP_A = 5.770780 / SS  # 4*log2(e) / 256
EXP_B = 60.0 + 0.3 - 5.770780 * C0  # e5m2 bias 60, +0.3 truncation recenter
NWARM = 100           # HAM warmup matmuls (bridge ~9us DMA startup)

_compiled_cache = {}


def _build():
    from contextlib import ExitStack
    from concourse import bacc, tile, mybir, masks

    f32 = mybir.dt.float32
    bf = mybir.dt.bfloat16
    fp8e4 = mybir.dt.float8e4
    fp8e5 = mybir.dt.float8e5
    u8 = mybir.dt.uint8

    nc = bacc.Bacc("TRN2", target_bir_lowering=False, debug=False, num_devices=NCORES)

    x8_d = nc.dram_tensor("x8p", [128, KC, 256], fp8e4, kind="ExternalInput")
    xq_d = nc.dram_tensor("xqp", [128, 16, 256], f32, kind="ExternalInput")
    wq_d = nc.dram_tensor("wq8p", [128, 256], fp8e4, kind="ExternalInput")
    wk_d = nc.dram_tensor("wk8p", [128, 256], fp8e4, kind="ExternalInput")
    bq_d = nc.dram_tensor("bq_col", [128, 1], f32, kind="ExternalInput")
    bk_d = nc.dram_tensor("bk_col", [128, 1], f32, kind="ExternalInput")
    w2_d = nc.dram_tensor("w2", [256, 256], bf, kind="ExternalInput")
    out_d = nc.dram_tensor("out", [NQ, C], f32, kind="ExternalOutput")

    Exp = mybir.ActivationFunctionType.Exp
    Ident = mybir.ActivationFunctionType.Identity
    Add = mybir.AluOpType.add
    Mult = mybir.AluOpType.mult
    DR = mybir.MatmulPerfMode.DoubleRow

    with tile.TileContext(nc) as tc:
        with ExitStack() as ctx:
            const = ctx.enter_context(tc.tile_pool(name="const", bufs=1))
            big = ctx.enter_context(tc.tile_pool(name="big", bufs=1))
            expp = ctx.enter_context(tc.tile_pool(name="expp", bufs=8))
            small = ctx.enter_context(tc.tile_pool(name="small", bufs=2))
            ps_s = ctx.enter_context(tc.tile_pool(name="ps_s", bufs=2, space="PSUM"))
            ps_g = ctx.enter_context(tc.tile_pool(name="ps_g", bufs=1, space="PSUM"))
            ps_d = ctx.enter_context(tc.tile_pool(name="ps_d", bufs=1, space="PSUM"))
            ps_e = ctx.enter_context(tc.tile_pool(name="ps_e", bufs=1, space="PSUM"))

            # ---- identity + consts via gpsimd (no DMA dependency) ----
            ident = const.tile([128, 128], fp8e4, tag="ident")
            masks.make_identity(nc, ident[:])
            ones8 = const.tile([128, 32], fp8e5, tag="ones8")
            nc.gpsimd.memset(ones8[:], 1.0)
            ones1 = const.tile([128, 1], bf, tag="ones1")
            nc.gpsimd.memset(ones1[:], 1.0)
            negc0 = const.tile([128, 1], f32, tag="negc0")
            nc.gpsimd.memset(negc0[:], -C0)

            # ---- input DMAs: x8 split across both HWDGE rings, fat descriptors ----
            x8sb = big.tile([128, KC, 256], fp8e4, tag="x8sb")
            xqg = big.tile([128, 16, 256], f32, tag="xqg")
            wqsb = const.tile([128, 2, 128], fp8e4, tag="wqsb")
            wksb = const.tile([128, 2, 128], fp8e4, tag="wksb")
            w2sb = const.tile([128, 2, 256], bf, tag="w2sb")
            bqc = const.tile([128, 1], f32, tag="bqc")
            bkc = const.tile([128, 1], f32, tag="bkc")

            nc.sync.dma_start(out=x8sb[:, 0:8, :], in_=x8_d[:, 0:8, :])
            nc.scalar.dma_start(out=x8sb[:, 8:16, :], in_=x8_d[:, 8:16, :])
            nc.sync.dma_start(out=x8sb[:, 16:24, :], in_=x8_d[:, 16:24, :])
            nc.scalar.dma_start(out=x8sb[:, 24:32, :], in_=x8_d[:, 24:32, :])
            nc.scalar.dma_start(out=wqsb[:], in_=wq_d[:].rearrange("p (j m) -> p j m", j=2))
            nc.scalar.dma_start(out=wksb[:], in_=wk_d[:].rearrange("p (j m) -> p j m", j=2))
            nc.scalar.dma_start(out=w2sb[:, 0, :], in_=w2_d[0:128, :])
            nc.scalar.dma_start(out=w2sb[:, 1, :], in_=w2_d[128:256, :])
            nc.scalar.dma_start(out=bqc[:], in_=bq_d[:])
            nc.scalar.dma_start(out=bkc[:], in_=bk_d[:])
            nc.sync.dma_start(out=xqg[:, 0:8, :], in_=xq_d[:, 0:8, :])
            nc.sync.dma_start(out=xqg[:, 8:16, :], in_=xq_d[:, 8:16, :])

            # ---- HAM warmup: dense dummy matmuls while DMAs land ----
            wmt = ps_s.tile([128, 2, 512], f32, tag="s", name="warm")
            for i in range(NWARM):
                nc.tensor.matmul(wmt[:, 0, 0:128], ident[:], ident[:], start=True, stop=True)
            # pre-load ACT exp table
            dumm = const.tile([128, 1], f32, tag="dumm")
            nc.scalar.activation(dumm[:], negc0[:], Exp, bias=negc0[:])

            # ---- phase A: x8 -> xT8 (channel-major), via identity matmuls ----
            xT8 = big.tile([128, 2, N], fp8e4, tag="xT8")  # [:, h, :]: ch 128h..128h+127
            for t in range(16):
                ta, tb = 2 * t, 2 * t + 1
                pt = ps_s.tile([128, 2, 512], f32, tag="s")
                nc.tensor.matmul(pt[:, 0, 0:128], x8sb[:, ta, 0:128], ident[:], start=True, stop=True)
                nc.tensor.matmul(pt[:, 0, 128:256], x8sb[:, tb, 0:128], ident[:], start=True, stop=True)
                nc.tensor.matmul(pt[:, 1, 0:128], x8sb[:, ta, 128:256], ident[:], start=True, stop=True)
                nc.tensor.matmul(pt[:, 1, 128:256], x8sb[:, tb, 128:256], ident[:], start=True, stop=True)
                dst = xT8[:, :, 256 * t : 256 * t + 256]
                src = pt[:, :, 0:256]
                if t % 2 == 0:
                    nc.vector.tensor_copy(dst, src)
                else:
                    nc.scalar.copy(dst, src)

            # ---- phase B: qT/kT replicated x4 along partitions, DoubleRow ----
            qT = big.tile([128, NQ], bf, tag="qT")
            kT = big.tile([128, N], bf, tag="kT")
            for s in range(NQ // 512):
                pq = ps_s.tile([128, 2, 512], f32, tag="s")
                nc.tensor.matmul(pq[:, 0, :], wqsb[:], xT8[:, :, 512 * s : 512 * s + 512],
                                 start=True, stop=True, perf_mode=DR)
                dst = qT[:, 512 * s : 512 * s + 512]
                if s % 2 == 0:
                    nc.vector.tensor_scalar(dst, pq[:, 0, :], bqc[:], None, Add)
                else:
                    nc.scalar.activation(dst, pq[:, 0, :], Ident, bias=bqc[:])
            for s in range(N // 512):
                pk = ps_s.tile([128, 2, 512], f32, tag="s")
                nc.tensor.matmul(pk[:, 0, :], wksb[:], xT8[:, :, 512 * s : 512 * s + 512],
                                 start=True, stop=True, perf_mode=DR)
                dst = kT[:, 512 * s : 512 * s + 512]
                if s % 2 == 0:
                    nc.vector.tensor_scalar(dst, pk[:, 0, :], bkc[:], None, Add)
                else:
                    nc.scalar.activation(dst, pk[:, 0, :], Ident, bias=bkc[:])

            # ---- phase C: flat pipeline over 32 steps of 4 key chunks ----
            sts = {}
            ets = {}
            gtile = {}
            dtile = {}
            ottile = {}

            def produce(s):
                g, t = divmod(s, NSTEP)
                if t == 0:
                    gtile[g] = ps_g.tile([128, 2, 512], f32, tag="g", name=f"g{g}")
                    dtile[g] = ps_d.tile([128, 512], f32, tag="d", name=f"d{g}")
                    ottile[g] = small.tile([128, 4, 256], f32, tag="ot", name=f"ot{g}")
                sA = ps_s.tile([128, 2, 512], f32, tag="s", name=f"sA{s}")
                sB = ps_s.tile([128, 2, 512], f32, tag="s", name=f"sB{s}")
                for i in range(4):
                    m = 4 * t + i
                    dst = (sA if i < 2 else sB)[:, i % 2, :]
                    nc.tensor.matmul(
                        dst,
                        kT[32 * i : 32 * i + 32, 128 * m : 128 * m + 128],
                        qT[32 * i : 32 * i + 32, 512 * g : 512 * g + 512],
                        start=True,
                        stop=True,
                        tile_position=(32 * i, 0),
                    )
                etA = expp.tile([128, 2, 512], fp8e5, tag="e", name=f"eA{s}")
                etB = expp.tile([128, 2, 512], fp8e5, tag="e", name=f"eB{s}")
                nc.scalar.activation(etA[:], sA[:], Exp, bias=negc0[:], scale=1.0 / SS)
                nc.vector.tensor_scalar(
                    etB[:].bitcast(u8), sB[:], EXP_A, EXP_B, Mult, Add
                )
                sts[s] = (sA, sB)
                ets[s] = (etA, etB)

            def consume(s):
                g, t = divmod(s, NSTEP)
                etA, etB = ets.pop(s)
                sts.pop(s)
                gp = gtile[g]
                dp = dtile[g]
                for pa, et in ((0, etA), (1, etB)):
                    pp = 2 * t + pa
                    for h in range(2):
                        nc.tensor.matmul(
                            gp[:, h, :],
                            x8sb[:, 4 * t + 2 * pa : 4 * t + 2 * pa + 2, 128 * h : 128 * h + 128],
                            et[:],
                            start=(pp == 0),
                            stop=(pp == 2 * NSTEP - 1),
                            perf_mode=DR,
                        )
                for j in range(4):
                    et = (etA if j < 2 else etB)
                    nc.tensor.matmul(
                        dp[32 * j : 32 * j + 32, :],
                        ones8[:],
                        et[:, j % 2, :],
                        start=(t == 0),
                        stop=(t == NSTEP - 1),
                        tile_position=(0, 32 * j),
                    )

            def group_epilogue(g):
                gp = gtile.pop(g)
                dp = dtile.pop(g)
                ott = ottile.pop(g)
                gsb = small.tile([128, 2, 512], bf, tag="gsb")
                nc.vector.tensor_copy(gsb[:, 0, :], gp[:, 0, :])
                nc.scalar.copy(gsb[:, 1, :], gp[:, 1, :])
                dsb = small.tile([128, 512], bf, tag="dsb")
                nc.vector.tensor_copy(dsb[:], dp[:])
                er = ps_e.tile([128, 512], f32, tag="er", name=f"er{g}")
                for b in range(4):
                    nc.tensor.matmul(
                        er[:, 256 + b : 257 + b], dsb[:, 128 * b : 128 * b + 128],
                        ones1[:], start=True, stop=True,
                    )
                rec = small.tile([128, 4], f32, tag="recs")
                nc.vector.reciprocal(rec[:], er[:, 256:260])
                for b in range(4):
                    ep = er[:, 0:256]
                    nc.tensor.matmul(ep, gsb[:, 0, 128 * b : 128 * b + 128], w2sb[:, 0, :], start=True, stop=False)
                    nc.tensor.matmul(ep, gsb[:, 1, 128 * b : 128 * b + 128], w2sb[:, 1, :], start=False, stop=True)
                    nc.vector.scalar_tensor_tensor(
                        ott[:, b, :], ep, rec[:, b : b + 1], xqg[:, 4 * g + b, :], Mult, Add
                    )
                out_r = out_d[:].rearrange("(t p) c -> p t c", p=128)
                nc.sync.dma_start(out=out_r[:, 4 * g : 4 * g + 4, :], in_=ott[:])

            for s in range(NG * NSTEP + PIPE):
                if s >= PIPE:
                    sc = s - PIPE
                    consume(sc)
                    if sc % NSTEP == NSTEP - 1:
                        group_epilogue(sc // NSTEP)
                if s < NG * NSTEP:
                    produce(s)

    nc.compile()
    return nc


def _get_compiled():
    if "v4" not in _compiled_cache:
        _compiled_cache["v4"] = _build()
    return _compiled_cache["v4"]


def _prep(x, wq, bq, wk, bk, wv, bv, wo, bo):
    xf = np.ascontiguousarray(np.asarray(x, dtype=np.float32)).reshape(B, N, C)
    wq = np.asarray(wq, np.float32)
    bq = np.asarray(bq, np.float32)
    wk = np.asarray(wk, np.float32)
    bk = np.asarray(bk, np.float32)
    wv = np.asarray(wv, np.float32)
    bv = np.asarray(bv, np.float32)
    wo = np.asarray(wo, np.float32)
    bo = np.asarray(bo, np.float32)

    scale = np.float32(1.0 / np.sqrt(np.float32(D)))
    # wq,wk stored e4m3 at x16 scale each (S comes out x256; exp rescales),
    # replicated x4 along d, then packed [128, 2, 128] for DoubleRow.
    wq_rep = np.tile(wq * (scale * 16.0), (1, 4)).astype(E4M3)  # [256, 128]
    wk_rep = np.tile(wk * 16.0, (1, 4)).astype(E4M3)
    wq8p = np.ascontiguousarray(wq_rep.reshape(2, 128, 128).transpose(1, 0, 2).reshape(128, 256))
    wk8p = np.ascontiguousarray(wk_rep.reshape(2, 128, 128).transpose(1, 0, 2).reshape(128, 256))
    bq_col = np.ascontiguousarray(np.tile(bq * (scale * 16.0), 4)[:, None]).astype(np.float32)
    bk_col = np.ascontiguousarray(np.tile(bk * 16.0, 4)[:, None]).astype(np.float32)
    w2 = np.ascontiguousarray(32.0 * (wv @ wo)).astype(BF16)  # x32 folds denom replication
    r0 = (bv @ wo + bo).astype(np.float32)  # residual-folded constant bias row

    in_maps = []
    for core in range(NCORES):
        b, h = divmod(core, 2)
        if h == 0:
            xo = xf[b]
        else:
            xo = np.concatenate([xf[b, NQ:], xf[b, :NQ]], 0)
        # partition-major layouts: [p, t, c] with 8/16KB contiguous per partition
        x8p = np.ascontiguousarray(xo.astype(E4M3).reshape(KC, 128, 256).transpose(1, 0, 2))
        xqp = np.ascontiguousarray(
            (xo[:NQ] + r0[None, :]).reshape(16, 128, 256).transpose(1, 0, 2)
        )
        in_maps.append(
            {
                "x8p": x8p,
                "xqp": xqp,
                "wq8p": wq8p,
                "wk8p": wk8p,
                "bq_col": bq_col,
                "bk_col": bk_col,
                "w2": w2,
            }
        )
    return in_maps


def _gather(results):
    out = np.empty((B, N, C), np.float32)
    for core in range(NCORES):
        b, h = divmod(core, 2)
        out[b, NQ * h : NQ * (h + 1)] = results[core]["out"]
    return out.reshape(B, HH, WW, C)


def kernel(x, wq, bq, wk, bk, wv, bv, wo, bo):
    from concourse.bass_utils import run_bass_kernel_spmd

    in_maps = _prep(x, wq, bq, wk, bk, wv, bv, wo, bo)
    nc = _get_compiled()
    res = run_bass_kernel_spmd(nc, in_maps, core_ids=list(range(NCORES)))
    return _gather(res.results)


def _ensure_ntff_hook():
    """The agent image's antenv stub lacks axon_hooks; synthesize it so
    run_bass_kernel_spmd(trace=True) can NTFF-profile via libaxon_pjrt."""
    import types

    try:
        from antenv.axon_hooks import get_axon_ntff_profile_hook  # noqa: F401
        return
    except ImportError:
        pass
    import antenv
    from trn_agent_boot.trn_boot import _ntff_profile_via_ctypes

    mod = types.ModuleType("antenv.axon_hooks")
    state = {"h": _ntff_profile_via_ctypes("/opt/axon/libaxon_pjrt.so")}
    mod.get_axon_ntff_profile_hook = lambda: state["h"]
    mod.set_axon_ntff_profile_hook = lambda h: state.__setitem__("h", h)
    sys.modules["antenv.axon_hooks"] = mod
    antenv.axon_hooks = mod


def run_traced(inputs, **kw):
    """For test.py: run with NTFF profiling; returns (output, BassKernelResults)."""
    from concourse.bass_utils import run_bass_kernel_spmd

    _ensure_ntff_hook()

    in_maps = _prep(**inputs)
    nc = _get_compiled()
    res = run_bass_kernel_spmd(nc, in_maps, core_ids=list(range(NCORES)), trace=True, **kw)
    return _gather(res.results), res


# revision 18
# speedup vs baseline: 1.3139x; 1.0008x over previous
"""Trainium2 Bass kernel for nn_AttentionBlock (B=4, H=W=64, C=256, D=32).

Sharding: 8 shards = 4 samples x 2 query-halves. Each core gets the full
sample's rows (reordered so its 2048 query rows come first), computes K for
all 4096 keys, and attention for its 2048 queries. No collectives.

v4 algorithm (projection folding + fp8 end-to-end):
  out = x + (1/d) * (G^T @ W2),  W2 = 32 * wv @ wo   (host precompute)
  G[c,q] = sum_k x8[k,c] * E8[k,q]   (fp8 DoubleRow matmuls, contraction 256)
  E8 = fp8e5m2(exp(S/256 - 2)),  S = K Q^T scores at x256 scale (wq,wk
       stored e4m3 with x16 scale each; compensated in the exp scale)
  d[q] = sum_k E8[k,q]       (col-packed ones matmuls + transpose matmul)
exp computed two ways in parallel: ACT true exp -> e5m2, and DVE integer
bit-trick (Schraudolph in e5m2 space: bits = (5.7708/256)*S + 48.76).
S matmuls (contraction D=32) use 4-way PE row-tiling via tile_position.
Q/K projections use DoubleRow. HAM warmup matmuls bridge the ~9us DMA
startup dead zone so the PE clock is at 2.4GHz when real work arrives.
Host supplies x8/xq32 in partition-major layout for fat DMA descriptors.

Self-contained: hardcodes shapes, imports only /opt/trn_rl_repo concourse.
"""

import sys

if "/opt/trn_rl_repo" not in sys.path:
    sys.path.insert(0, "/opt/trn_rl_repo")

import numpy as np
import ml_dtypes

BF16 = ml_dtypes.bfloat16
E4M3 = ml_dtypes.float8_e4m3
E5M2 = ml_dtypes.float8_e5m2

# Problem constants
B, HH, WW, C = 4, 64, 64, 256
D = 32
N = HH * WW           # 4096 keys per sample
NQ = N // 2           # 2048 queries per core
NCORES = 8
KC = N // 128         # 32 key chunks
NG = NQ // 512        # 4 query groups of 512 per core
NSTEP = 8             # 4-chunk steps per query group (32 chunks / 4)
PIPE = 4              # consume s - PIPE

C0 = 2.0              # exp shift: weights = exp(S - C0), cancels in softmax
SS = 256.0            # scores arrive at x256 scale (wq,wk each x16 in fp8)
EXP_A = 5.770780 / SS  # 4*log2(e) / 256
EXP_B = 60.0 + 0.3 - 5.770780 * C0  # e5m2 bias 60, +0.3 truncation recenter
NWARM = 120           # HAM warmup matmuls (bridge ~9us DMA startup)

_compiled_cache = {}


def _build():
    from contextlib import ExitStack
    from concourse import bacc, tile, mybir, masks

    f32 = mybir.dt.float32
    bf = mybir.dt.bfloat16
    fp8e4 = mybir.dt.float8e4
    fp8e5 = mybir.dt.float8e5
    u8 = mybir.dt.uint8

    nc = bacc.Bacc("TRN2", target_bir_lowering=False, debug=False, num_devices=NCORES)

    x8_d = nc.dram_tensor("x8p", [128, KC, 256], fp8e4, kind="ExternalInput")
    xq_d = nc.dram_tensor("xqp", [128, 16, 256], f32, kind="ExternalInput")
    wq_d = nc.dram_tensor("wq8p", [128, 256], fp8e4, kind="ExternalInput")
    wk_d = nc.dram_tensor("wk8p", [128, 256], fp8e4, kind="ExternalInput")
    bq_d = nc.dram_tensor("bq_col", [128, 1], f32, kind="ExternalInput")
    bk_d = nc.dram_tensor("bk_col", [128, 1], f32, kind="ExternalInput")
    w2_d = nc.dram_tensor("w2", [256, 256], bf, kind="ExternalInput")
    out_d = nc.dram_tensor("out", [NQ, C], f32, kind="ExternalOutput")

    Exp = mybir.ActivationFunctionType.Exp
    Ident = mybir.ActivationFunctionType.Identity
    Add = mybir.AluOpType.add
    Mult = mybir.AluOpType.mult
    DR = mybir.MatmulPerfMode.DoubleRow

    with tile.TileContext(nc) as tc:
        with ExitStack() as ctx:
            const = ctx.enter_context(tc.tile_pool(name="const", bufs=1))
            big = ctx.enter_context(tc.tile_pool(name="big", bufs=1))
            expp = ctx.enter_context(tc.tile_pool(name="expp", bufs=12))
            small = ctx.enter_context(tc.tile_pool(name="small", bufs=2))
            ps_s = ctx.enter_context(tc.tile_pool(name="ps_s", bufs=2, space="PSUM"))
            ps_g = ctx.enter_context(tc.tile_pool(name="ps_g", bufs=1, space="PSUM"))
            ps_d = ctx.enter_context(tc.tile_pool(name="ps_d", bufs=1, space="PSUM"))
            ps_e = ctx.enter_context(tc.tile_pool(name="ps_e", bufs=1, space="PSUM"))

            # ---- identity + consts via gpsimd (no DMA dependency) ----
            ident = const.tile([128, 128], fp8e4, tag="ident")
            masks.make_identity(nc, ident[:])
            ones8 = const.tile([128, 32], fp8e5, tag="ones8")
            nc.gpsimd.memset(ones8[:], 1.0)
            ones1 = const.tile([128, 1], bf, tag="ones1")
            nc.gpsimd.memset(ones1[:], 1.0)
            negc0 = const.tile([128, 1], f32, tag="negc0")
            nc.gpsimd.memset(negc0[:], -C0)

            # ---- input DMAs: x8 split across both HWDGE rings, fat descriptors ----
            x8sb = big.tile([128, KC, 256], fp8e4, tag="x8sb")
            xqg = big.tile([128, 16, 256], f32, tag="xqg")
            wqsb = const.tile([128, 2, 128], fp8e4, tag="wqsb")
            wksb = const.tile([128, 2, 128], fp8e4, tag="wksb")
            w2sb = const.tile([128, 2, 256], bf, tag="w2sb")
            bqc = const.tile([128, 1], f32, tag="bqc")
            bkc = const.tile([128, 1], f32, tag="bkc")

            nc.sync.dma_start(out=x8sb[:, 0:2, :], in_=x8_d[:, 0:2, :])
            nc.sync.dma_start(out=x8sb[:, 2:8, :], in_=x8_d[:, 2:8, :])
            nc.scalar.dma_start(out=x8sb[:, 8:16, :], in_=x8_d[:, 8:16, :])
            nc.sync.dma_start(out=x8sb[:, 16:24, :], in_=x8_d[:, 16:24, :])
            nc.scalar.dma_start(out=x8sb[:, 24:32, :], in_=x8_d[:, 24:32, :])
            nc.scalar.dma_start(out=wqsb[:], in_=wq_d[:].rearrange("p (j m) -> p j m", j=2))
            nc.scalar.dma_start(out=wksb[:], in_=wk_d[:].rearrange("p (j m) -> p j m", j=2))
            nc.scalar.dma_start(out=w2sb[:, 0, :], in_=w2_d[0:128, :])
            nc.scalar.dma_start(out=w2sb[:, 1, :], in_=w2_d[128:256, :])
            nc.scalar.dma_start(out=bqc[:], in_=bq_d[:])
            nc.scalar.dma_start(out=bkc[:], in_=bk_d[:])
            nc.sync.dma_start(out=xqg[:, 0:8, :], in_=xq_d[:, 0:8, :])
            nc.sync.dma_start(out=xqg[:, 8:16, :], in_=xq_d[:, 8:16, :])

            # ---- HAM warmup: dense dummy matmuls while DMAs land ----
            wmt = ps_s.tile([128, 2, 512], f32, tag="s", name="warm")
            for i in range(NWARM):
                nc.tensor.matmul(wmt[:, 0, 0:128], ident[:], ident[:], start=True, stop=True)
            # pre-load ACT exp table
            dumm = const.tile([128, 1], f32, tag="dumm")
            nc.scalar.activation(dumm[:], negc0[:], Exp, bias=negc0[:])

            # ---- phase A: x8 -> xT8 (channel-major), via identity matmuls ----
            xT8 = big.tile([128, 2, N], fp8e4, tag="xT8")  # [:, h, :]: ch 128h..128h+127
            for t in range(16):
                ta, tb = 2 * t, 2 * t + 1
                pt = ps_s.tile([128, 2, 512], f32, tag="s")
                nc.tensor.matmul(pt[:, 0, 0:128], x8sb[:, ta, 0:128], ident[:], start=True, stop=True)
                nc.tensor.matmul(pt[:, 0, 128:256], x8sb[:, tb, 0:128], ident[:], start=True, stop=True)
                nc.tensor.matmul(pt[:, 1, 0:128], x8sb[:, ta, 128:256], ident[:], start=True, stop=True)
                nc.tensor.matmul(pt[:, 1, 128:256], x8sb[:, tb, 128:256], ident[:], start=True, stop=True)
                dst = xT8[:, :, 256 * t : 256 * t + 256]
                src = pt[:, :, 0:256]
                if t % 2 == 0:
                    nc.vector.tensor_copy(dst, src)
                else:
                    nc.scalar.copy(dst, src)

            # ---- phase B: qT/kT replicated x4 along partitions, DoubleRow ----
            qT = big.tile([128, NQ], bf, tag="qT")
            kT = big.tile([128, N], bf, tag="kT")
            for s in range(NQ // 512):
                pq = ps_s.tile([128, 2, 512], f32, tag="s")
                nc.tensor.matmul(pq[:, 0, :], wqsb[:], xT8[:, :, 512 * s : 512 * s + 512],
                                 start=True, stop=True, perf_mode=DR)
                dst = qT[:, 512 * s : 512 * s + 512]
                if s % 2 == 0:
                    nc.vector.tensor_scalar(dst, pq[:, 0, :], bqc[:], None, Add)
                else:
                    nc.scalar.activation(dst, pq[:, 0, :], Ident, bias=bqc[:])
            for s in range(N // 512):
                pk = ps_s.tile([128, 2, 512], f32, tag="s")
                nc.tensor.matmul(pk[:, 0, :], wksb[:], xT8[:, :, 512 * s : 512 * s + 512],
                                 start=True, stop=True, perf_mode=DR)
                dst = kT[:, 512 * s : 512 * s + 512]
                if s % 2 == 0:
                    nc.vector.tensor_scalar(dst, pk[:, 0, :], bkc[:], None, Add)
                else:
                    nc.scalar.activation(dst, pk[:, 0, :], Ident, bias=bkc[:])

            # ---- phase C: flat pipeline over 32 steps of 4 key chunks ----
            sts = {}
            ets = {}
            gtile = {}
            dtile = {}
            ottile = {}

            def produce(s):
                g, t = divmod(s, NSTEP)
                if t == 0:
                    gtile[g] = ps_g.tile([128, 2, 512], f32, tag="g", name=f"g{g}")
                    dtile[g] = ps_d.tile([128, 512], f32, tag="d", name=f"d{g}")
                    ottile[g] = small.tile([128, 4, 256], f32, tag="ot", name=f"ot{g}")
                if s < PIPE + 2:
                    # filler matmuls: keep the PE dense (HAM warm) through
                    # pipeline fill while exp results are still in flight
                    fil = ps_e.tile([128, 256], f32, tag="er", name=f"fil{s}")
                    for i in range(16):
                        nc.tensor.matmul(fil[:, 0:128], ident[:], ident[:], start=True, stop=True)
                sA = ps_s.tile([128, 2, 512], f32, tag="s", name=f"sA{s}")
                sB = ps_s.tile([128, 2, 512], f32, tag="s", name=f"sB{s}")
                for i in range(4):
                    m = 4 * t + i
                    dst = (sA if i < 2 else sB)[:, i % 2, :]
                    nc.tensor.matmul(
                        dst,
                        kT[32 * i : 32 * i + 32, 128 * m : 128 * m + 128],
                        qT[32 * i : 32 * i + 32, 512 * g : 512 * g + 512],
                        start=True,
                        stop=True,
                        tile_position=(32 * i, 0),
                    )
                etA = expp.tile([128, 2, 512], fp8e5, tag="e", name=f"eA{s}")
                etB = expp.tile([128, 2, 512], fp8e5, tag="e", name=f"eB{s}")
                nc.scalar.activation(etA[:], sA[:], Exp, bias=negc0[:], scale=1.0 / SS)
                nc.vector.tensor_scalar(
                    etB[:].bitcast(u8), sB[:], EXP_A, EXP_B, Mult, Add
                )
                sts[s] = (sA, sB)
                ets[s] = (etA, etB)

            def consume(s):
                g, t = divmod(s, NSTEP)
                etA, etB = ets.pop(s)
                sts.pop(s)
                gp = gtile[g]
                dp = dtile[g]
                for pa, et in ((0, etA), (1, etB)):
                    pp = 2 * t + pa
                    for h in range(2):
                        nc.tensor.matmul(
                            gp[:, h, :],
                            x8sb[:, 4 * t + 2 * pa : 4 * t + 2 * pa + 2, 128 * h : 128 * h + 128],
                            et[:],
                            start=(pp == 0),
                            stop=(pp == 2 * NSTEP - 1),
                            perf_mode=DR,
                        )
                for j in range(4):
                    et = (etA if j < 2 else etB)
                    nc.tensor.matmul(
                        dp[32 * j : 32 * j + 32, :],
                        ones8[:],
                        et[:, j % 2, :],
                        start=(t == 0),
                        stop=(t == NSTEP - 1),
                        tile_position=(0, 32 * j),
                    )

            def group_epilogue(g):
                gp = gtile.pop(g)
                dp = dtile.pop(g)
                ott = ottile.pop(g)
                # dsb cast first: it heads the rec chain gating all 4 blocks
                dsb = small.tile([128, 512], bf, tag="dsb")
                nc.vector.tensor_copy(dsb[:], dp[:])
                # gsb cast split per q-block across vector+scalar for latency
                gsb = small.tile([128, 2, 512], bf, tag="gsb")
                for b in range(4):
                    dst = gsb[:, :, 128 * b : 128 * b + 128]
                    src = gp[:, :, 128 * b : 128 * b + 128]
                    if b % 2 == 0:
                        nc.scalar.copy(dst, src)
                    else:
                        nc.vector.tensor_copy(dst, src)
                # rec matmuls reuse the denom bank (tag d) after dp is drained
                rp = ps_d.tile([128, 512], f32, tag="d", name=f"rp{g}")
                for b in range(4):
                    nc.tensor.matmul(
                        rp[:, b : b + 1], dsb[:, 128 * b : 128 * b + 128],
                        ones1[:], start=True, stop=True,
                    )
                rec = small.tile([128, 4], f32, tag="recs")
                nc.vector.reciprocal(rec[:], rp[:, 0:4])
                for b in range(4):
                    ep = ps_e.tile([128, 256], f32, tag="er", name=f"er{g}_{b}")
                    nc.tensor.matmul(ep[:], gsb[:, 0, 128 * b : 128 * b + 128], w2sb[:, 0, :], start=True, stop=False)
                    nc.tensor.matmul(ep[:], gsb[:, 1, 128 * b : 128 * b + 128], w2sb[:, 1, :], start=False, stop=True)
                    nc.vector.scalar_tensor_tensor(
                        ott[:, b, :], ep[:], rec[:, b : b + 1], xqg[:, 4 * g + b, :], Mult, Add
                    )
                out_r = out_d[:].rearrange("(t p) c -> p t c", p=128)
                nc.sync.dma_start(out=out_r[:, 4 * g : 4 * g + 4, :], in_=ott[:])

            for s in range(NG * NSTEP + PIPE):
                if s >= PIPE:
                    sc = s - PIPE
                    consume(sc)
                    if sc % NSTEP == NSTEP - 1:
                        group_epilogue(sc // NSTEP)
                if s < NG * NSTEP:
                    produce(s)

    nc.compile()
    return nc


def _get_compiled():
    if "v4" not in _compiled_cache:
        _compiled_cache["v4"] = _build()
    return _compiled_cache["v4"]


def _prep(x, wq, bq, wk, bk, wv, bv, wo, bo):
    xf = np.ascontiguousarray(np.asarray(x, dtype=np.float32)).reshape(B, N, C)
    wq = np.asarray(wq, np.float32)
    bq = np.asarray(bq, np.float32)
    wk = np.asarray(wk, np.float32)
    bk = np.asarray(bk, np.float32)
    wv = np.asarray(wv, np.float32)
    bv = np.asarray(bv, np.float32)
    wo = np.asarray(wo, np.float32)
    bo = np.asarray(bo, np.float32)

    scale = np.float32(1.0 / np.sqrt(np.float32(D)))
    # wq,wk stored e4m3 at x16 scale each (S comes out x256; exp rescales),
    # replicated x4 along d, then packed [128, 2, 128] for DoubleRow.
    wq_rep = np.tile(wq * (scale * 16.0), (1, 4)).astype(E4M3)  # [256, 128]
    wk_rep = np.tile(wk * 16.0, (1, 4)).astype(E4M3)
    wq8p = np.ascontiguousarray(wq_rep.reshape(2, 128, 128).transpose(1, 0, 2).reshape(128, 256))
    wk8p = np.ascontiguousarray(wk_rep.reshape(2, 128, 128).transpose(1, 0, 2).reshape(128, 256))
    bq_col = np.ascontiguousarray(np.tile(bq * (scale * 16.0), 4)[:, None]).astype(np.float32)
    bk_col = np.ascontiguousarray(np.tile(bk * 16.0, 4)[:, None]).astype(np.float32)
    w2 = np.ascontiguousarray(32.0 * (wv @ wo)).astype(BF16)  # x32 folds denom replication
    r0 = (bv @ wo + bo).astype(np.float32)  # residual-folded constant bias row

    in_maps = []
    for core in range(NCORES):
        b, h = divmod(core, 2)
        if h == 0:
            xo = xf[b]
        else:
            xo = np.concatenate([xf[b, NQ:], xf[b, :NQ]], 0)
        # partition-major layouts: [p, t, c] with 8/16KB contiguous per partition
        x8p = np.ascontiguousarray(xo.astype(E4M3).reshape(KC, 128, 256).transpose(1, 0, 2))
        xqp = np.ascontiguousarray(
            (xo[:NQ] + r0[None, :]).reshape(16, 128, 256).transpose(1, 0, 2)
        )
        in_maps.append(
            {
                "x8p": x8p,
                "xqp": xqp,
                "wq8p": wq8p,
                "wk8p": wk8p,
                "bq_col": bq_col,
                "bk_col": bk_col,
                "w2": w2,
            }
        )
    return in_maps


def _gather(results):
    out = np.empty((B, N, C), np.float32)
    for core in range(NCORES):
        b, h = divmod(core, 2)
        out[b, NQ * h : NQ * (h + 1)] = results[core]["out"]
    return out.reshape(B, HH, WW, C)


def kernel(x, wq, bq, wk, bk, wv, bv, wo, bo):
    from concourse.bass_utils import run_bass_kernel_spmd

    in_maps = _prep(x, wq, bq, wk, bk, wv, bv, wo, bo)
    nc = _get_compiled()
    res = run_bass_kernel_spmd(nc, in_maps, core_ids=list(range(NCORES)))
    return _gather(res.results)


def _ensure_ntff_hook():
    """The agent image's antenv stub lacks axon_hooks; synthesize it so
    run_bass_kernel_spmd(trace=True) can NTFF-profile via libaxon_pjrt."""
    import types

    try:
        from antenv.axon_hooks import get_axon_ntff_profile_hook  # noqa: F401
        return
    except ImportError:
        pass
    import antenv
    from trn_agent_boot.trn_boot import _ntff_profile_via_ctypes

    mod = types.ModuleType("antenv.axon_hooks")
    state = {"h": _ntff_profile_via_ctypes("/opt/axon/libaxon_pjrt.so")}
    mod.get_axon_ntff_profile_hook = lambda: state["h"]
    mod.set_axon_ntff_profile_hook = lambda h: state.__setitem__("h", h)
    sys.modules["antenv.axon_hooks"] = mod
    antenv.axon_hooks = mod


def run_traced(inputs, **kw):
    """For test.py: run with NTFF profiling; returns (output, BassKernelResults)."""
    from concourse.bass_utils import run_bass_kernel_spmd

    _ensure_ntff_hook()

    in_maps = _prep(**inputs)
    nc = _get_compiled()
    res = run_bass_kernel_spmd(nc, in_maps, core_ids=list(range(NCORES)), trace=True, **kw)
    return _gather(res.results), res
